# revision 26
# baseline (speedup 1.0000x reference)
"""Causal multi-head self-attention with RoPE — Trainium2 Bass kernel.

Problem: B=2, S=2048, D=1024, H=16 heads, dk=64, fp32 in/out.
Sharding: 8 cores = 2 batches x 4 head-groups. Each core computes ONE batch
and FOUR heads (two head-pairs). Wq/Wk/Wv split column-wise (by head), Wo
row-wise; the host sums the 4 partial outputs per batch in fp32.

All matmul operands are bf16 (PSUM accumulation fp32): same PE rate as
fp32r but 4x faster weight loads (FWL) and 2x faster DVE elementwise.

Host-side prep: x -> x^T (bf16); Wq/Wk rows reordered within each head to
even-first/odd-second ("half-split") so RoPE on device becomes a
32-partition block-swap + elementwise ops (scores are invariant to a shared
permutation of q and k head dims); cos / sign-folded-sin tables (fp32).

Per-core device pipeline (emission interleaves the three streams so the
PE never idles long enough to re-throttle its clock):
  proj(jt):  xT chunks -> Q^T/K^T/V^T (dk on partitions, tokens free);
             RoPE via t1 = pp*cos, w2 = pp*spre, sh = blockswap32(w2)
             (GpSimd SWDGE SBUF->SBUF DMA), q/k = t1 + sh;
             V: PE-transpose to token-partition tiles with ones columns.
  attn(qt):  per k-tile group: scores^T for BOTH heads packed in one
             [128, 1024] PSUM tile (the two heads' matmuls use disjoint
             PE row groups and run concurrently), causal mask added via
             identity-matmul on diagonal tiles, ONE exp (ScalarE) per
             group covering both heads, PV accumulated into [65, 512]
             PSUM with a ones-column producing softmax denominators;
             normalize via reciprocal_approx_fast + gpsimd broadcast.
  outp(qt):  y tiles = sum_pairs un^T.T @ Wo_pair, cast bf16, DMA out.
"""

import sys

sys.path.insert(0, "/opt/trn_rl_repo")

import numpy as np
import ml_dtypes

import concourse.bass as bass
import concourse.tile as tile
import concourse.mybir as mybir
from concourse import bacc
from concourse.masks import make_identity

# ---------------------------------------------------------------- constants
B = 2
S = 2048
D = 1024
H = 16
DK = 64
THETA = 10000.0
NCORES = 8
P = 128
CH = D // P                 # 8 contraction chunks of 128
NQT = S // 512              # 4 query tiles of 512
NPR = 2                     # head pairs per core (4 heads = 2 pairs of 2)
MASK_NEG = -480.0           # pre-scale mask add; *0.125 => -60 in the exponent

BF16 = mybir.dt.bfloat16
F32 = mybir.dt.float32
NP_BF16 = ml_dtypes.bfloat16


def build_nc():
    """Build the per-core Bass program (SPMD: all cores run this, with
    per-core batch slice + weight slices in their input maps)."""
    nc = bacc.Bacc("TRN2", target_bir_lowering=False, debug=False)

    xT = nc.dram_tensor("xT", [D, S], BF16, kind="ExternalInput")
    wq = nc.dram_tensor("wq", [D, 2 * P], BF16, kind="ExternalInput")
    wk = nc.dram_tensor("wk", [D, 2 * P], BF16, kind="ExternalInput")
    wv = nc.dram_tensor("wv", [D, 2 * P], BF16, kind="ExternalInput")
    wo = nc.dram_tensor("wo", [2 * P, D], BF16, kind="ExternalInput")
    cosT = nc.dram_tensor("cosT", [P, S], F32, kind="ExternalInput")
    sinT = nc.dram_tensor("sinT", [P, S], F32, kind="ExternalInput")
    y = nc.dram_tensor("y", [S, D], BF16, kind="ExternalOutput")

    with tile.TileContext(nc) as tc:
        _emit(nc, tc, xT, wq, wk, wv, wo, cosT, sinT, y)
    nc.compile()
    return nc


def _emit(nc, tc, xT, wq, wk, wv, wo, cosT, sinT, y):
    from contextlib import ExitStack

    ctx = ExitStack()
    with ctx:
        # ------------------------------------------------ pools
        singles = ctx.enter_context(tc.tile_pool(name="singles", bufs=1))
        xp = ctx.enter_context(tc.tile_pool(name="xp", bufs=3))
        tabs = ctx.enter_context(tc.tile_pool(name="tabs", bufs=1))
        qkp = ctx.enter_context(tc.tile_pool(name="qkp", bufs=1))
        vp = ctx.enter_context(tc.tile_pool(name="vp", bufs=1))
        ropet = ctx.enter_context(tc.tile_pool(name="ropet", bufs=2))
        expp = ctx.enter_context(tc.tile_pool(name="expp", bufs=4))
        unp = ctx.enter_context(tc.tile_pool(name="unp", bufs=1))
        rrp = ctx.enter_context(tc.tile_pool(name="rrp", bufs=2))
        ysp = ctx.enter_context(tc.tile_pool(name="ysp", bufs=3))

        # PSUM: 8 banks total. sg 2 slots x [128,1024] = 4 banks;
        # ps_o (attention out + ones row) 2 slots x [65,512] = 2 banks;
        # "u" (proj / v-transpose / out-proj) 2 slots x [128,512] = 2.
        psA = ctx.enter_context(tc.tile_pool(name="psA", bufs=2, space="PSUM"))
        psB = ctx.enter_context(tc.tile_pool(name="psB", bufs=2, space="PSUM"))
        psC = ctx.enter_context(tc.tile_pool(name="psC", bufs=2, space="PSUM"))

        # ------------------------------------------------ constants
        # (memset/affine_select can't write bf16 directly; build in f32 and
        # round via a DVE copy)
        ident_f = ropet.tile([P, P], F32, tag="t1", name="ident_f")
        make_identity(nc, ident_f)
        ident = singles.tile([P, P], BF16)
        nc.vector.tensor_copy(ident[:], ident_f[:])

        # tri[r, c] = 1 if c >= r else 0 — multiplied onto the diagonal
        # 128-block of the exp tile to apply the causal mask (DVE mul is
        # cheaper than the identity-matmul mask add on the PE)
        tri_f = ropet.tile([P, P], F32, tag="w2", name="tri_f")
        nc.gpsimd.memset(tri_f[:], 1.0)
        nc.gpsimd.affine_select(
            out=tri_f[:],
            in_=tri_f[:],
            compare_op=mybir.AluOpType.is_ge,
            fill=0.0,
            base=0,
            pattern=[[1, P]],
            channel_multiplier=-1,
        )
        tri_sb = singles.tile([P, P], BF16)
        nc.vector.tensor_copy(tri_sb[:], tri_f[:])

        ones_f = ropet.tile([P, 1], F32, tag="sh", name="ones_f")
        nc.vector.memset(ones_f[:], 1.0)
        ones_sb = singles.tile([P, 1], BF16)
        nc.vector.tensor_copy(ones_sb[:], ones_f[:])

        # PE warm-up: ~3.5us of dummy matmuls while the input DMAs stream.
        # The HAM clock gate needs one busy 3.4us window to lift the PE
        # from 1.2 to 2.4 GHz; without this the first projections run at
        # half clock.
        warm_ps = psC.tile([P, P], F32, tag="u", name="warm_ps")
        for _ in range(30):
            nc.tensor.matmul(warm_ps[:], ident[:], ident[:],
                             start=True, stop=True)

        # Force the GpSimd ucode library containing partition_broadcast to
        # load NOW (overlaps the input-DMA wait). Without this the
        # UNLOAD_LIB/LOAD_LIB swap lands right before the first extraction
        # and stalls every engine ~7us mid-kernel.
        warm_bc = singles.tile([2, 1], F32)
        nc.gpsimd.partition_broadcast(warm_bc[:], ones_f[0:1, 0:1])

        # weights: [D, 256] -> per-pair SBUF [128, CH, 128] tiles;
        # wo [256, D] -> [128, 2, D]. Loaded in first-use order so the
        # first projection matmul starts as early as possible.
        w_dram = {"wq": wq, "wk": wk, "wv": wv}
        w_sbs = {nm: [singles.tile([P, CH, P], BF16, name=f"{nm}_sb{pr}")
                      for pr in range(NPR)] for nm in w_dram}

        def load_w(nm, pr):
            nc.sync.dma_start(
                w_sbs[nm][pr][:],
                w_dram[nm][:, pr * P:(pr + 1) * P].rearrange(
                    "(c p) m -> p c m", p=P),
            )

        wo_sb = singles.tile([P, NPR, D], BF16)
        load_w("wq", 0)

        # -------------------------------------------- load x^T, tables
        xc = {}
        cos_t, spre_t = [], []
        for jt in range(NQT):
            for c in range(CH):
                t = xp.tile([P, 512], BF16, tag=f"xc{c}", name=f"xc_{c}_{jt}")
                nc.sync.dma_start(
                    t[:], xT[c * P:(c + 1) * P, jt * 512:(jt + 1) * 512]
                )
                xc[(c, jt)] = t
            ct = tabs.tile([P, 512], F32, tag=f"cos{jt}", name=f"cos_{jt}")
            nc.sync.dma_start(ct[:], cosT[:, jt * 512:(jt + 1) * 512])
            cos_t.append(ct)
            st = tabs.tile([P, 512], F32, tag=f"spre{jt}", name=f"spre_{jt}")
            nc.sync.dma_start(st[:], sinT[:, jt * 512:(jt + 1) * 512])
            spre_t.append(st)
            if jt == 0:
                load_w("wq", 1)
                load_w("wk", 0)
                load_w("wk", 1)
                load_w("wv", 0)
                load_w("wv", 1)
                nc.sync.dma_start(
                    wo_sb[:], wo.ap().rearrange("(r p) d -> p r d", p=P)
                )

        q_t = [[qkp.tile([P, 512], BF16, tag=f"q{pr}_{jt}", name=f"q_{pr}_{jt}")
                for jt in range(NQT)] for pr in range(NPR)]
        k_t = [[qkp.tile([P, 512], BF16, tag=f"k{pr}_{jt}", name=f"k_{pr}_{jt}")
                for jt in range(NQT)] for pr in range(NPR)]
        v_jt = [[vp.tile([P, 4, 130], BF16, tag=f"v{pr}_{jt}", name=f"v_{pr}_{jt}")
                 for jt in range(NQT)] for pr in range(NPR)]

        un_all = {}

        # ---------------- emission units (closures); drained interleaved
        def proj_units(jt):
            """6 units: one per (nm, pr) projection of query-tile jt."""
            units = []
            for nm in ("wq", "wk", "wv"):
                for pr in range(NPR):
                    def u(nm=nm, pr=pr, jt=jt):
                        w_sb = w_sbs[nm]
                        pp = psC.tile([P, 512], F32, tag="u",
                                      name=f"pp_{nm}_{pr}_{jt}")
                        for c in range(CH):
                            nc.tensor.matmul(
                                pp[:],
                                w_sb[pr][:, c, :],
                                xc[(c, jt)][:],
                                start=(c == 0),
                                stop=(c == CH - 1),
                            )
                        if nm == "wv":
                            # copies on ScalarE: it is idle during the
                            # projection-heavy stretches and this keeps the
                            # DVE free for RoPE / normalization
                            vt = vp.tile([P, 512], BF16, tag="vt",
                                         name=f"vt_{pr}_{jt}")
                            nc.scalar.copy(vt[:], pp[:])
                            nc.vector.tensor_copy(
                                v_jt[pr][jt][:, :, 64::65],
                                ones_sb[:, 0:1].to_broadcast([P, 4, 2]),
                            )
                            pt = psC.tile([P, 512], BF16, tag="u",
                                          name=f"pvt_{pr}_{jt}")
                            for ti in range(4):
                                nc.tensor.transpose(
                                    pt[:, ti * P:(ti + 1) * P],
                                    vt[:, ti * P:(ti + 1) * P],
                                    ident[:],
                                )
                            nc.scalar.copy(
                                v_jt[pr][jt].rearrange(
                                    "p f (h c) -> p f h c", h=2)[:, :, :, 0:64],
                                pt.rearrange("p (f h c) -> p f h c", f=4, h=2),
                            )
                        else:
                            dst = q_t if nm == "wq" else k_t
                            # RoPE: dst = pp*cos + blockswap32(pp*spre)
                            t1 = ropet.tile([P, 512], BF16, tag="t1",
                                            name=f"t1_{nm}_{pr}_{jt}")
                            nc.vector.tensor_mul(t1[:], pp[:], cos_t[jt][:])
                            w2 = ropet.tile([P, 512], BF16, tag="w2",
                                            name=f"w2_{nm}_{pr}_{jt}")
                            nc.vector.tensor_mul(w2[:], pp[:], spre_t[jt][:])
                            sh = ropet.tile([P, 512], BF16, tag="sh",
                                            name=f"sh_{nm}_{pr}_{jt}")
                            for blk in range(4):
                                src_blk = blk ^ 1  # swap 32-blocks within 64
                                # SWDGE (GpSimd) SBUF->SBUF: keeps the
                                # Scalar engine free for exp
                                nc.gpsimd.dma_start(
                                    sh[blk * 32:(blk + 1) * 32, :],
                                    w2[src_blk * 32:(src_blk + 1) * 32, :],
                                )
                            nc.vector.tensor_add(dst[pr][jt][:], t1[:], sh[:])
                    units.append(u)
            return units

        def attn_units(qt):
            """Per (pr, kt) group: scores for both heads into one PSUM
            tile, one exp, PV accumulate. Extraction unit per pr."""
            qs = qt * 512
            nkt = qt * 4 + 4
            units = []
            for pr in range(NPR):
                ps_o = [
                    psB.tile([65, 512], F32, tag="o", name=f"po_{qt}_{pr}_{h}")
                    for h in range(2)
                ]
                for kt in range(nkt):
                    def u(kt=kt, pr=pr, qt=qt, qs=qs, nkt=nkt, ps_o=ps_o):
                        ks = kt * P
                        dlt = ks - qs
                        off = max(dlt, 0)
                        sg = psA.tile([P, 1024], F32, tag="s",
                                      name=f"sg_{qt}_{pr}_{kt}")
                        for h in range(2):
                            nc.tensor.matmul(
                                sg[:, h * 512 + off:(h + 1) * 512],
                                k_t[pr][ks // 512][h * 64:h * 64 + 64,
                                                   ks % 512:ks % 512 + P],
                                q_t[pr][qt][h * 64:h * 64 + 64, off:512],
                                start=True,
                                stop=True,
                            )
                        e = expp.tile([P, 1024], BF16, tag="e",
                                      name=f"e_{qt}_{pr}_{kt}")
                        if dlt < 0:
                            nc.scalar.activation(
                                e[:], sg[:],
                                mybir.ActivationFunctionType.Exp, scale=0.125,
                            )
                        else:
                            nc.scalar.activation(
                                e.rearrange("p (u c) -> p u c",
                                            u=2)[:, :, off:512],
                                sg.rearrange("p (u c) -> p u c",
                                             u=2)[:, :, off:512],
                                mybir.ActivationFunctionType.Exp, scale=0.125,
                            )
                            ev = e.rearrange("p (u c) -> p u c",
                                             u=2)[:, :, dlt:dlt + P]
                            nc.vector.tensor_mul(
                                ev, ev,
                                tri_sb.rearrange(
                                    "p (u c) -> p u c",
                                    u=1).to_broadcast([P, 2, P]),
                            )
                        for h in range(2):
                            nc.tensor.matmul(
                                ps_o[h][:, off:512],
                                v_jt[pr][kt // 4][:, kt % 4,
                                                  h * 65:h * 65 + 65],
                                e[:, h * 512 + off:(h + 1) * 512],
                                start=(kt == 0),
                                stop=(kt == nkt - 1),
                            )
                    units.append(u)

                def ext(qt=qt, pr=pr, ps_o=ps_o):
                    # normalize + stack pair: un [128 = 2x64 headdim, 512 tok]
                    un = unp.tile([P, 512], BF16, tag=f"un{pr}_{qt}",
                                  name=f"un_{qt}_{pr}")
                    rrs, rbs = [], []
                    for h in range(2):
                        den = rrp.tile([1, 512], F32, tag="den",
                                       name=f"den_{qt}_{pr}_{h}")
                        nc.vector.tensor_copy(den[0:1, :], ps_o[h][64:65, :])
                        rr = rrp.tile([1, 512], F32, tag="rr",
                                      name=f"rr_{qt}_{pr}_{h}")
                        nc.vector.reciprocal_approx_fast(rr[0:1, :], den[0:1, :])
                        rrs.append(rr)
                    for h in range(2):
                        rb = rrp.tile([64, 512], F32, tag="rb",
                                      name=f"rb_{qt}_{pr}_{h}")
                        nc.gpsimd.partition_broadcast(rb[:], rrs[h][0:1, :])
                        rbs.append(rb)
                    for h in range(2):
                        nc.vector.tensor_mul(
                            un[h * 64:(h + 1) * 64, :], ps_o[h][0:64, :],
                            rbs[h][:]
                        )
                    un_all[(qt, pr)] = un
                units.append(ext)
            return units

        def outp_units(qt):
            """8 units: y tiles for query-tile qt."""
            units = []
            for ti in range(4):
                for n in range(2):
                    def u(qt=qt, ti=ti, n=n):
                        un0 = un_all[(qt, 0)]
                        un1 = un_all[(qt, 1)]
                        tt = qt * 4 + ti
                        yp = psC.tile([P, 512], F32, tag="u",
                                      name=f"yp_{tt}_{n}")
                        nc.tensor.matmul(
                            yp[:],
                            un0[:, ti * P:(ti + 1) * P],
                            wo_sb[:, 0, n * 512:(n + 1) * 512],
                            start=True,
                            stop=False,
                        )
                        nc.tensor.matmul(
                            yp[:],
                            un1[:, ti * P:(ti + 1) * P],
                            wo_sb[:, 1, n * 512:(n + 1) * 512],
                            start=False,
                            stop=True,
                        )
                        ys = ysp.tile([P, 512], BF16, tag="ys",
                                      name=f"ys_{tt}_{n}")
                        # in the late phases ScalarE has slack (exp winding
                        # down); alternating copy engines halves the
                        # "u"-slot pacing
                        if qt >= NQT - 2 and (ti + n) % 2 == 0:
                            nc.scalar.copy(ys[:], yp[:])
                        else:
                            nc.vector.tensor_copy(ys[:], yp[:])
                        nc.sync.dma_start(
                            y[tt * P:(tt + 1) * P, n * 512:(n + 1) * 512],
                            ys[:],
                        )
                    units.append(u)
            return units

        def drain(lists):
            """Round-robin the unit lists proportionally so each stream
            spreads across the phase (keeps all engines fed)."""
            lists = [list(l) for l in lists if l]
            total = sum(len(l) for l in lists)
            done = [0] * len(lists)
            for i in range(total):
                # pick the list most behind its proportional schedule
                best, bscore = None, None
                for j, l in enumerate(lists):
                    if done[j] < len(l):
                        score = done[j] / len(l)
                        if bscore is None or score < bscore:
                            best, bscore = j, score
                lists[best][done[best]]()
                done[best] += 1

        # ---------------- phases. Attention query-tiles are CONCATENATED
        # into one ordered chain (qt n+1's PV may wait on qt n's extraction
        # via PSUM slot reuse — emitting them in order avoids blocking the
        # PE queue); proj / out-proj streams are mixed in proportionally to
        # fill the PE while the ScalarE chews exp.
        drain([proj_units(0) + proj_units(1)])
        drain([attn_units(0) + attn_units(1),
               proj_units(2) + proj_units(3)])
        drain([attn_units(2) + attn_units(3),
               outp_units(0) + outp_units(1) + outp_units(2)])

        # Keep the PE clock warm through the final extraction lull (the
        # DVE/GpSimd normalization chain leaves the PE idle ~5us right
        # before the last out-projections, which would re-throttle it to
        # 1.2 GHz). The scores PSUM banks are free by now.
        warm_tail = psA.tile([P, 512], F32, tag="s", name="warm_tail")
        for _ in range(24):
            nc.tensor.matmul(warm_tail[:, 0:P], ident[:], ident[:],
                             start=True, stop=True)
        drain([outp_units(3)])


# ------------------------------------------------------------------ host side

_PERM_HS = np.concatenate([np.arange(0, DK, 2), np.arange(1, DK, 2)])


def host_inputs(x, token_positions, Wq, Wk, Wv, Wo):
    """Build the per-core device input maps (core c: batch c//4, heads
    4*(c%4) .. 4*(c%4)+3)."""
    x = np.asarray(x, dtype=np.float32)
    tp = np.asarray(token_positions)
    Wq = np.asarray(Wq, dtype=np.float32)
    Wk = np.asarray(Wk, dtype=np.float32)
    Wv = np.asarray(Wv, dtype=np.float32)
    Wo = np.asarray(Wo, dtype=np.float32)

    xT = np.ascontiguousarray(x.transpose(0, 2, 1)).astype(NP_BF16)  # [B,D,S]

    # RoPE tables in the half-split + swapped-sin formulation
    inv_freq = (1.0 / (THETA ** (np.arange(0, DK, 2, dtype=np.float32) / DK))).astype(
        np.float32
    )  # [32]
    ang = tp.astype(np.float32)[:, None, :] * inv_freq[np.arange(P) % 32][None, :, None]
    cosT = np.cos(ang).astype(np.float32)  # [B, 128, S]
    sgn = np.where((np.arange(P) // 32) % 2 == 0, 1.0, -1.0).astype(np.float32)
    sinT = (np.sin(ang) * sgn[None, :, None]).astype(np.float32)

    in_maps = []
    for c in range(NCORES):
        bidx = c // 4
        heads = [4 * (c % 4) + i for i in range(4)]
        rows_hs = np.concatenate([h * DK + _PERM_HS for h in heads])   # q/k rows
        rows_pl = np.concatenate([h * DK + np.arange(DK) for h in heads])
        in_maps.append(
            {
                "xT": xT[bidx],
                "wq": np.ascontiguousarray(Wq[rows_hs].T).astype(NP_BF16),
                "wk": np.ascontiguousarray(Wk[rows_hs].T).astype(NP_BF16),
                "wv": np.ascontiguousarray(Wv[rows_pl].T).astype(NP_BF16),
                "wo": np.ascontiguousarray(Wo[:, rows_pl].T).astype(NP_BF16),
                "cosT": cosT[bidx],
                "sinT": sinT[bidx],
            }
        )
    return in_maps


_NC_CACHE = None


def kernel(x, token_positions, Wq, Wk, Wv, Wo, _want_results=False, **run_kwargs):
    """Full-input, full-output entry point. Shards across 8 NeuronCores."""
    global _NC_CACHE
    from concourse.bass_utils import run_bass_kernel_spmd

    in_maps = host_inputs(x, token_positions, Wq, Wk, Wv, Wo)
    if _NC_CACHE is None:
        _NC_CACHE = build_nc()
    res = run_bass_kernel_spmd(
        _NC_CACHE, in_maps, core_ids=list(range(NCORES)), **run_kwargs
    )
    out = np.zeros((B, S, D), dtype=np.float32)
    for c, r in enumerate(res.results):
        out[c // 4] += np.asarray(r["y"], dtype=np.float32)
    if _want_results:
        return out, res
    return out


# revision 31
# speedup vs baseline: 1.0228x; 1.0228x over previous
"""Causal multi-head self-attention with RoPE — Trainium2 Bass kernel.

Problem: B=2, S=2048, D=1024, H=16 heads, dk=64, fp32 in/out.
Sharding: 8 cores = 2 batches x 4 head-groups. Each core computes ONE batch
and FOUR heads (two head-pairs). Wq/Wk/Wv split column-wise (by head), Wo
row-wise; the host sums the 4 partial outputs per batch in fp32.

All matmul operands are bf16 (PSUM accumulation fp32): same PE rate as
fp32r but 4x faster weight loads (FWL) and 2x faster DVE elementwise.

Host-side prep: x -> x^T (bf16); Wq/Wk rows reordered within each head to
even-first/odd-second ("half-split") so RoPE on device becomes a
32-partition block-swap + elementwise ops (scores are invariant to a shared
permutation of q and k head dims); cos / sign-folded-sin tables (fp32).

Per-core device pipeline (emission interleaves the three streams so the
PE never idles long enough to re-throttle its clock):
  proj(jt):  xT chunks -> Q^T/K^T/V^T (dk on partitions, tokens free);
             RoPE via t1 = pp*cos, w2 = pp*spre, sh = blockswap32(w2)
             (GpSimd SWDGE SBUF->SBUF DMA), q/k = t1 + sh;
             V: PE-transpose to token-partition tiles with ones columns.
  attn(qt):  per k-tile group: scores^T for BOTH heads packed in one
             [128, 1024] PSUM tile (the two heads' matmuls use disjoint
             PE row groups and run concurrently), causal mask added via
             identity-matmul on diagonal tiles, ONE exp (ScalarE) per
             group covering both heads, PV accumulated into [65, 512]
             PSUM with a ones-column producing softmax denominators;
             normalize via reciprocal_approx_fast + gpsimd broadcast.
  outp(qt):  y tiles = sum_pairs un^T.T @ Wo_pair, cast bf16, DMA out.
"""

import sys

sys.path.insert(0, "/opt/trn_rl_repo")

import numpy as np
import ml_dtypes

import concourse.bass as bass
import concourse.tile as tile
import concourse.mybir as mybir
from concourse import bacc
from concourse.masks import make_identity

# ---------------------------------------------------------------- constants
B = 2
S = 2048
D = 1024
H = 16
DK = 64
THETA = 10000.0
NCORES = 8
P = 128
CH = D // P                 # 8 contraction chunks of 128
NQT = S // 512              # 4 query tiles of 512
NPR = 2                     # head pairs per core (4 heads = 2 pairs of 2)
MASK_NEG = -480.0           # pre-scale mask add; *0.125 => -60 in the exponent

BF16 = mybir.dt.bfloat16
F32 = mybir.dt.float32
NP_BF16 = ml_dtypes.bfloat16


def build_nc():
    """Build the per-core Bass program (SPMD: all cores run this, with
    per-core batch slice + weight slices in their input maps)."""
    nc = bacc.Bacc("TRN2", target_bir_lowering=False, debug=False)

    xT = nc.dram_tensor("xT", [D, S], BF16, kind="ExternalInput")
    wq = nc.dram_tensor("wq", [D, 2 * P], BF16, kind="ExternalInput")
    wk = nc.dram_tensor("wk", [D, 2 * P], BF16, kind="ExternalInput")
    wv = nc.dram_tensor("wv", [D, 2 * P], BF16, kind="ExternalInput")
    wo = nc.dram_tensor("wo", [2 * P, D], BF16, kind="ExternalInput")
    cosT = nc.dram_tensor("cosT", [P, S], F32, kind="ExternalInput")
    sinT = nc.dram_tensor("sinT", [P, S], F32, kind="ExternalInput")
    y = nc.dram_tensor("y", [S, D], BF16, kind="ExternalOutput")

    with tile.TileContext(nc) as tc:
        _emit(nc, tc, xT, wq, wk, wv, wo, cosT, sinT, y)
    nc.compile()
    return nc


def _emit(nc, tc, xT, wq, wk, wv, wo, cosT, sinT, y):
    from contextlib import ExitStack

    ctx = ExitStack()
    with ctx:
        # ------------------------------------------------ pools
        singles = ctx.enter_context(tc.tile_pool(name="singles", bufs=1))
        xp = ctx.enter_context(tc.tile_pool(name="xp", bufs=3))
        tabs = ctx.enter_context(tc.tile_pool(name="tabs", bufs=1))
        qkp = ctx.enter_context(tc.tile_pool(name="qkp", bufs=1))
        vp = ctx.enter_context(tc.tile_pool(name="vp", bufs=1))
        ropet = ctx.enter_context(tc.tile_pool(name="ropet", bufs=2))
        expp = ctx.enter_context(tc.tile_pool(name="expp", bufs=4))
        unp = ctx.enter_context(tc.tile_pool(name="unp", bufs=1))
        rrp = ctx.enter_context(tc.tile_pool(name="rrp", bufs=2))
        ysp = ctx.enter_context(tc.tile_pool(name="ysp", bufs=3))

        # PSUM: 8 banks total. sg 2 slots x [128,1024] = 4 banks;
        # ps_o (attention out + ones row) 2 slots x [65,512] = 2 banks;
        # "u" (proj / v-transpose / out-proj) 2 slots x [128,512] = 2.
        psA = ctx.enter_context(tc.tile_pool(name="psA", bufs=2, space="PSUM"))
        psB = ctx.enter_context(tc.tile_pool(name="psB", bufs=2, space="PSUM"))
        psC = ctx.enter_context(tc.tile_pool(name="psC", bufs=2, space="PSUM"))

        # ------------------------------------------------ constants
        # (memset/affine_select can't write bf16 directly; build in f32 and
        # round via a DVE copy)
        ident_f = ropet.tile([P, P], F32, tag="t1", name="ident_f")
        make_identity(nc, ident_f)
        ident = singles.tile([P, P], BF16)
        nc.vector.tensor_copy(ident[:], ident_f[:])

        # tri[r, c] = 1 if c >= r else 0 — multiplied onto the diagonal
        # 128-block of the exp tile to apply the causal mask (DVE mul is
        # cheaper than the identity-matmul mask add on the PE)
        tri_f = ropet.tile([P, P], F32, tag="w2", name="tri_f")
        nc.gpsimd.memset(tri_f[:], 1.0)
        nc.gpsimd.affine_select(
            out=tri_f[:],
            in_=tri_f[:],
            compare_op=mybir.AluOpType.is_ge,
            fill=0.0,
            base=0,
            pattern=[[1, P]],
            channel_multiplier=-1,
        )
        tri_sb = singles.tile([P, P], BF16)
        nc.vector.tensor_copy(tri_sb[:], tri_f[:])

        ones_f = ropet.tile([P, 1], F32, tag="sh", name="ones_f")
        nc.vector.memset(ones_f[:], 1.0)
        ones_sb = singles.tile([P, 1], BF16)
        nc.vector.tensor_copy(ones_sb[:], ones_f[:])

        # PE warm-up: ~3.5us of dummy matmuls while the input DMAs stream.
        # The HAM clock gate needs one busy 3.4us window to lift the PE
        # from 1.2 to 2.4 GHz; without this the first projections run at
        # half clock.
        warm_ps = psC.tile([P, P], F32, tag="u", name="warm_ps")
        for _ in range(30):
            nc.tensor.matmul(warm_ps[:], ident[:], ident[:],
                             start=True, stop=True)

        # Force the GpSimd ucode library containing partition_broadcast to
        # load NOW (overlaps the input-DMA wait). Without this the
        # UNLOAD_LIB/LOAD_LIB swap lands right before the first extraction
        # and stalls every engine ~7us mid-kernel.
        warm_bc = singles.tile([2, 1], F32)
        nc.gpsimd.partition_broadcast(warm_bc[:], ones_f[0:1, 0:1])

        # weights: [D, 256] -> per-pair SBUF [128, CH, 128] tiles;
        # wo [256, D] -> [128, 2, D]. Loaded in first-use order so the
        # first projection matmul starts as early as possible.
        w_dram = {"wq": wq, "wk": wk, "wv": wv}
        w_sbs = {nm: [singles.tile([P, CH, P], BF16, name=f"{nm}_sb{pr}")
                      for pr in range(NPR)] for nm in w_dram}

        def load_w(nm, pr):
            nc.sync.dma_start(
                w_sbs[nm][pr][:],
                w_dram[nm][:, pr * P:(pr + 1) * P].rearrange(
                    "(c p) m -> p c m", p=P),
            )

        wo_sb = singles.tile([P, NPR, D], BF16)
        load_w("wq", 0)

        # -------------------------------------------- load x^T, tables
        xc = {}
        cos_t, spre_t = [], []
        for jt in range(NQT):
            for c in range(CH):
                t = xp.tile([P, 512], BF16, tag=f"xc{c}", name=f"xc_{c}_{jt}")
                nc.sync.dma_start(
                    t[:], xT[c * P:(c + 1) * P, jt * 512:(jt + 1) * 512]
                )
                xc[(c, jt)] = t
            ct = tabs.tile([P, 512], F32, tag=f"cos{jt}", name=f"cos_{jt}")
            nc.sync.dma_start(ct[:], cosT[:, jt * 512:(jt + 1) * 512])
            cos_t.append(ct)
            st = tabs.tile([P, 512], F32, tag=f"spre{jt}", name=f"spre_{jt}")
            nc.sync.dma_start(st[:], sinT[:, jt * 512:(jt + 1) * 512])
            spre_t.append(st)
            if jt == 0:
                load_w("wq", 1)
                load_w("wk", 0)
                load_w("wk", 1)
                load_w("wv", 0)
                load_w("wv", 1)
                nc.sync.dma_start(
                    wo_sb[:], wo.ap().rearrange("(r p) d -> p r d", p=P)
                )

        q_t = [[qkp.tile([P, 512], BF16, tag=f"q{pr}_{jt}", name=f"q_{pr}_{jt}")
                for jt in range(NQT)] for pr in range(NPR)]
        k_t = [[qkp.tile([P, 512], BF16, tag=f"k{pr}_{jt}", name=f"k_{pr}_{jt}")
                for jt in range(NQT)] for pr in range(NPR)]
        v_jt = [[vp.tile([P, 4, 130], BF16, tag=f"v{pr}_{jt}", name=f"v_{pr}_{jt}")
                 for jt in range(NQT)] for pr in range(NPR)]

        un_all = {}

        # ---------------- emission units (closures); drained interleaved
        def proj_units(jt):
            """6 units: one per (nm, pr) projection of query-tile jt."""
            units = []
            for nm in ("wq", "wk", "wv"):
                for pr in range(NPR):
                    def u(nm=nm, pr=pr, jt=jt):
                        w_sb = w_sbs[nm]
                        pp = psC.tile([P, 512], F32, tag="u",
                                      name=f"pp_{nm}_{pr}_{jt}")
                        for c in range(CH):
                            nc.tensor.matmul(
                                pp[:],
                                w_sb[pr][:, c, :],
                                xc[(c, jt)][:],
                                start=(c == 0),
                                stop=(c == CH - 1),
                            )
                        if nm == "wv":
                            # copies on ScalarE: it is idle during the
                            # projection-heavy stretches and this keeps the
                            # DVE free for RoPE / normalization
                            vt = vp.tile([P, 512], BF16, tag="vt",
                                         name=f"vt_{pr}_{jt}")
                            nc.scalar.copy(vt[:], pp[:])
                            nc.vector.tensor_copy(
                                v_jt[pr][jt][:, :, 64::65],
                                ones_sb[:, 0:1].to_broadcast([P, 4, 2]),
                            )
                            pt = psC.tile([P, 512], BF16, tag="u",
                                          name=f"pvt_{pr}_{jt}")
                            for ti in range(4):
                                nc.tensor.transpose(
                                    pt[:, ti * P:(ti + 1) * P],
                                    vt[:, ti * P:(ti + 1) * P],
                                    ident[:],
                                )
                            nc.scalar.copy(
                                v_jt[pr][jt].rearrange(
                                    "p f (h c) -> p f h c", h=2)[:, :, :, 0:64],
                                pt.rearrange("p (f h c) -> p f h c", f=4, h=2),
                            )
                        else:
                            dst = q_t if nm == "wq" else k_t
                            # RoPE: dst = pp*cos + blockswap32(pp*spre)
                            t1 = ropet.tile([P, 512], BF16, tag="t1",
                                            name=f"t1_{nm}_{pr}_{jt}")
                            nc.vector.tensor_mul(t1[:], pp[:], cos_t[jt][:])
                            w2 = ropet.tile([P, 512], BF16, tag="w2",
                                            name=f"w2_{nm}_{pr}_{jt}")
                            nc.vector.tensor_mul(w2[:], pp[:], spre_t[jt][:])
                            sh = ropet.tile([P, 512], BF16, tag="sh",
                                            name=f"sh_{nm}_{pr}_{jt}")
                            for blk in range(4):
                                src_blk = blk ^ 1  # swap 32-blocks within 64
                                # SWDGE (GpSimd) SBUF->SBUF: keeps the
                                # Scalar engine free for exp
                                nc.gpsimd.dma_start(
                                    sh[blk * 32:(blk + 1) * 32, :],
                                    w2[src_blk * 32:(src_blk + 1) * 32, :],
                                )
                            nc.vector.tensor_add(dst[pr][jt][:], t1[:], sh[:])
                    units.append(u)
            return units

        def attn_units(qt):
            """Per (pr, kt) group: scores for both heads into one PSUM
            tile, one exp, PV accumulate. Extraction unit per pr."""
            qs = qt * 512
            nkt = qt * 4 + 4
            units = []
            for pr in range(NPR):
                ps_o = [
                    psB.tile([65, 512], F32, tag="o", name=f"po_{qt}_{pr}_{h}")
                    for h in range(2)
                ]
                for kt in range(nkt):
                    def u(kt=kt, pr=pr, qt=qt, qs=qs, nkt=nkt, ps_o=ps_o):
                        ks = kt * P
                        dlt = ks - qs
                        off = max(dlt, 0)
                        sg = psA.tile([P, 1024], F32, tag="s",
                                      name=f"sg_{qt}_{pr}_{kt}")
                        for h in range(2):
                            nc.tensor.matmul(
                                sg[:, h * 512 + off:(h + 1) * 512],
                                k_t[pr][ks // 512][h * 64:h * 64 + 64,
                                                   ks % 512:ks % 512 + P],
                                q_t[pr][qt][h * 64:h * 64 + 64, off:512],
                                start=True,
                                stop=True,
                            )
                        e = expp.tile([P, 1024], BF16, tag="e",
                                      name=f"e_{qt}_{pr}_{kt}")
                        if dlt < 0:
                            nc.scalar.activation(
                                e[:], sg[:],
                                mybir.ActivationFunctionType.Exp, scale=0.125,
                            )
                        else:
                            nc.scalar.activation(
                                e.rearrange("p (u c) -> p u c",
                                            u=2)[:, :, off:512],
                                sg.rearrange("p (u c) -> p u c",
                                             u=2)[:, :, off:512],
                                mybir.ActivationFunctionType.Exp, scale=0.125,
                            )
                            ev = e.rearrange("p (u c) -> p u c",
                                             u=2)[:, :, dlt:dlt + P]
                            nc.vector.tensor_mul(
                                ev, ev,
                                tri_sb.rearrange(
                                    "p (u c) -> p u c",
                                    u=1).to_broadcast([P, 2, P]),
                            )
                        for h in range(2):
                            nc.tensor.matmul(
                                ps_o[h][:, off:512],
                                v_jt[pr][kt // 4][:, kt % 4,
                                                  h * 65:h * 65 + 65],
                                e[:, h * 512 + off:(h + 1) * 512],
                                start=(kt == 0),
                                stop=(kt == nkt - 1),
                            )
                    units.append(u)

                def ext(qt=qt, pr=pr, ps_o=ps_o):
                    # normalize + stack pair: un [128 = 2x64 headdim, 512 tok]
                    un = unp.tile([P, 512], BF16, tag=f"un{pr}_{qt}",
                                  name=f"un_{qt}_{pr}")
                    rrs, rbs = [], []
                    for h in range(2):
                        den = rrp.tile([1, 512], F32, tag="den",
                                       name=f"den_{qt}_{pr}_{h}")
                        nc.vector.tensor_copy(den[0:1, :], ps_o[h][64:65, :])
                        rr = rrp.tile([1, 512], F32, tag="rr",
                                      name=f"rr_{qt}_{pr}_{h}")
                        nc.vector.reciprocal_approx_fast(rr[0:1, :], den[0:1, :])
                        rrs.append(rr)
                    for h in range(2):
                        rb = rrp.tile([64, 512], F32, tag="rb",
                                      name=f"rb_{qt}_{pr}_{h}")
                        nc.gpsimd.partition_broadcast(rb[:], rrs[h][0:1, :])
                        rbs.append(rb)
                    for h in range(2):
                        nc.vector.tensor_mul(
                            un[h * 64:(h + 1) * 64, :], ps_o[h][0:64, :],
                            rbs[h][:]
                        )
                    un_all[(qt, pr)] = un
                units.append(ext)
            return units

        def outp_units(qt):
            """8 units: y tiles for query-tile qt."""
            units = []
            for ti in range(4):
                for n in range(2):
                    def u(qt=qt, ti=ti, n=n):
                        un0 = un_all[(qt, 0)]
                        un1 = un_all[(qt, 1)]
                        tt = qt * 4 + ti
                        yp = psC.tile([P, 512], F32, tag="u",
                                      name=f"yp_{tt}_{n}")
                        nc.tensor.matmul(
                            yp[:],
                            un0[:, ti * P:(ti + 1) * P],
                            wo_sb[:, 0, n * 512:(n + 1) * 512],
                            start=True,
                            stop=False,
                        )
                        nc.tensor.matmul(
                            yp[:],
                            un1[:, ti * P:(ti + 1) * P],
                            wo_sb[:, 1, n * 512:(n + 1) * 512],
                            start=False,
                            stop=True,
                        )
                        ys = ysp.tile([P, 512], BF16, tag="ys",
                                      name=f"ys_{tt}_{n}")
                        # in the final phase ScalarE is idle (no exp left);
                        # alternating copy engines halves the "u"-slot pacing
                        if qt == NQT - 1 and (ti + n) % 2 == 0:
                            nc.scalar.copy(ys[:], yp[:])
                        else:
                            nc.vector.tensor_copy(ys[:], yp[:])
                        nc.sync.dma_start(
                            y[tt * P:(tt + 1) * P, n * 512:(n + 1) * 512],
                            ys[:],
                        )
                    units.append(u)
            return units

        def drain(lists):
            """Round-robin the unit lists proportionally so each stream
            spreads across the phase (keeps all engines fed)."""
            lists = [list(l) for l in lists if l]
            total = sum(len(l) for l in lists)
            done = [0] * len(lists)
            for i in range(total):
                # pick the list most behind its proportional schedule
                best, bscore = None, None
                for j, l in enumerate(lists):
                    if done[j] < len(l):
                        score = done[j] / len(l)
                        if bscore is None or score < bscore:
                            best, bscore = j, score
                lists[best][done[best]]()
                done[best] += 1

        # ---------------- phases. Attention query-tiles are CONCATENATED
        # into one ordered chain (qt n+1's PV may wait on qt n's extraction
        # via PSUM slot reuse — emitting them in order avoids blocking the
        # PE queue); proj / out-proj streams are mixed in proportionally to
        # fill the PE while the ScalarE chews exp. attn(0) starts right
        # after proj(0) so the exp stream (the ScalarE pacer) begins ~12us
        # earlier than with a proj(0)+proj(1) prologue phase.
        drain([proj_units(0)])
        drain([attn_units(0), proj_units(1)])
        drain([attn_units(1), proj_units(2) + proj_units(3)])
        drain([attn_units(2) + attn_units(3),
               outp_units(0) + outp_units(1) + outp_units(2)])
        drain([outp_units(3)])


# ------------------------------------------------------------------ host side

_PERM_HS = np.concatenate([np.arange(0, DK, 2), np.arange(1, DK, 2)])


def host_inputs(x, token_positions, Wq, Wk, Wv, Wo):
    """Build the per-core device input maps (core c: batch c//4, heads
    4*(c%4) .. 4*(c%4)+3)."""
    x = np.asarray(x, dtype=np.float32)
    tp = np.asarray(token_positions)
    Wq = np.asarray(Wq, dtype=np.float32)
    Wk = np.asarray(Wk, dtype=np.float32)
    Wv = np.asarray(Wv, dtype=np.float32)
    Wo = np.asarray(Wo, dtype=np.float32)

    xT = np.ascontiguousarray(x.transpose(0, 2, 1)).astype(NP_BF16)  # [B,D,S]

    # RoPE tables in the half-split + swapped-sin formulation
    inv_freq = (1.0 / (THETA ** (np.arange(0, DK, 2, dtype=np.float32) / DK))).astype(
        np.float32
    )  # [32]
    ang = tp.astype(np.float32)[:, None, :] * inv_freq[np.arange(P) % 32][None, :, None]
    cosT = np.cos(ang).astype(np.float32)  # [B, 128, S]
    sgn = np.where((np.arange(P) // 32) % 2 == 0, 1.0, -1.0).astype(np.float32)
    sinT = (np.sin(ang) * sgn[None, :, None]).astype(np.float32)

    in_maps = []
    for c in range(NCORES):
        bidx = c // 4
        heads = [4 * (c % 4) + i for i in range(4)]
        rows_hs = np.concatenate([h * DK + _PERM_HS for h in heads])   # q/k rows
        rows_pl = np.concatenate([h * DK + np.arange(DK) for h in heads])
        in_maps.append(
            {
                "xT": xT[bidx],
                "wq": np.ascontiguousarray(Wq[rows_hs].T).astype(NP_BF16),
                "wk": np.ascontiguousarray(Wk[rows_hs].T).astype(NP_BF16),
                "wv": np.ascontiguousarray(Wv[rows_pl].T).astype(NP_BF16),
                "wo": np.ascontiguousarray(Wo[:, rows_pl].T).astype(NP_BF16),
                "cosT": cosT[bidx],
                "sinT": sinT[bidx],
            }
        )
    return in_maps


_NC_CACHE = None


def kernel(x, token_positions, Wq, Wk, Wv, Wo, _want_results=False, **run_kwargs):
    """Full-input, full-output entry point. Shards across 8 NeuronCores."""
    global _NC_CACHE
    from concourse.bass_utils import run_bass_kernel_spmd

    in_maps = host_inputs(x, token_positions, Wq, Wk, Wv, Wo)
    if _NC_CACHE is None:
        _NC_CACHE = build_nc()
    res = run_bass_kernel_spmd(
        _NC_CACHE, in_maps, core_ids=list(range(NCORES)), **run_kwargs
    )
    out = np.zeros((B, S, D), dtype=np.float32)
    for c, r in enumerate(res.results):
        out[c // 4] += np.asarray(r["y"], dtype=np.float32)
    if _want_results:
        return out, res
    return out
